# revision 10
# baseline (speedup 1.0000x reference)
"""STPAN (spatio-temporal attention net) Trainium2 kernel.

Strategy: pure data-parallel over batch (B=16 -> 2 per core on 8 cores).
Everything for one batch element fits in SBUF, so the whole network runs
fused on-chip per core with no collectives; only the x_data/id slices go in
and the [2, 12, 307] output comes out.

Host-side precomputation restructures the algebra (all exact, fp32-safe):
  * embedding: the time/day/node/pos components of the big concat-matmuls
    are folded into per-batch and per-t biases / small fused weights
    (low-rank through T=12), eliminating the 5H and 4H concat GEMMs.
  * memory module: argmin_m KL(q_n || p_m) == argmax_m (ms @ emb - ent)[m]
    because the log-sum-exp term is constant in m and sum_h ms[m,h] = 1.
    No exp/log needed on device; ties broken first-index via a cumsum trick.
  * attention: qk values are tiny (<0.5 scaled) so softmax needs no
    max-subtraction; exp'd scores are consumed in [key, query] layout and
    the denominator is produced by an extra all-ones column appended to V.
"""

import sys

sys.path.insert(0, "/opt/trn_rl_repo")
sys.path.insert(0, "/opt/trn_rl_repo/concourse")

from contextlib import ExitStack

import numpy as np

import concourse.bass as bass
import concourse.tile as tile
from concourse import bacc, mybir
from concourse.bass_utils import run_bass_kernel_spmd
from concourse.masks import make_identity

F32 = mybir.dt.float32
I32 = mybir.dt.int32
AF = mybir.ActivationFunctionType
ALU = mybir.AluOpType
AX = mybir.AxisListType

B, T, N, H, HEAD, M, P, OUT = 16, 12, 307, 64, 4, 20, 2, 12
NCORES = 8
BPC = B // NCORES  # batches per core
CH = [(0, 128), (128, 256), (256, 307)]  # node-dim chunks


# --------------------------------------------------------------------------
# host-side parameter restructuring
# --------------------------------------------------------------------------

def _prep_mem(mp, pfx, out):
    m1 = np.asarray(mp["m1"], np.float64)
    e = np.exp(m1 - m1.max(-1, keepdims=True))
    ms = e / e.sum(-1, keepdims=True)
    logms = m1 - m1.max(-1, keepdims=True) - np.log(e.sum(-1, keepdims=True))
    ent = (ms * logms).sum(-1)
    out[pfx + "_c1T"] = np.ascontiguousarray(np.asarray(mp["c1_w"], np.float32).T)
    out[pfx + "_c1b"] = np.asarray(mp["c1_b"], np.float32).reshape(H, 1)
    out[pfx + "_msT"] = np.ascontiguousarray(ms.astype(np.float32).T)  # [64, 20]
    out[pfx + "_negent"] = (-ent).astype(np.float32).reshape(M, 1)
    out[pfx + "_m1"] = np.asarray(mp["m1"], np.float32)  # [20, 64] lhsT
    out[pfx + "_c2T"] = np.ascontiguousarray(np.asarray(mp["c2_w"], np.float32).T)  # [128, 64]
    out[pfx + "_c2b"] = np.asarray(mp["c2_b"], np.float32).reshape(H, 1)


def _prep_params(params):
    f = np.float32
    pp = {}
    ep = {k: np.asarray(v, f) for k, v in params["emb"].items()}
    gp = params["gma"]

    l1_w = ep["l1_w"]  # [64, 320]
    pp["time_emb"] = ep["time_emb"]  # [288, 64]
    pp["day_emb"] = ep["day_emb"]    # [7, 64]
    pp["td_lw_r"] = np.ascontiguousarray(l1_w[:, 0:128].T)  # [128, 64]
    # fused per-t weight: l1_w_ier @ dense_w[t] -> lhsT [12, 64]
    dense_w = ep["dense_w"]  # [T, H, T]
    pp["fused_w"] = np.stack(
        [np.ascontiguousarray((l1_w[:, 128:192] @ dense_w[t]).T) for t in range(T)],
        axis=1,
    ).astype(f)  # [12(k), T, 64(m)]
    pp["r_node"] = (l1_w[:, 192:256] @ ep["node_emb"].T).astype(f)  # [64, 307]
    pp["l1_bt"] = np.ascontiguousarray(
        (
            ep["l1_b"][None, :]
            + ep["pos_emb"] @ l1_w[:, 256:320].T
            + ep["dense_b"] @ l1_w[:, 128:192].T
        ).T
    ).astype(f)  # [64, 12]

    c2_w = ep["c2_w"]  # [64, 256]
    pp["td_lw_p"] = np.ascontiguousarray(c2_w[:, 0:128].T)  # [128, 64]
    pp["p_w_x"] = np.ascontiguousarray((c2_w[:, 128:192] @ ep["c1_w"]).T)  # [12, 64]
    pp["p_node"] = (
        c2_w[:, 192:256] @ ep["node_emb"].T
        + (ep["c2_b"] + c2_w[:, 128:192] @ ep["c1_b"])[:, None]
    ).astype(f)  # [64, 307]

    _prep_mem({k: np.asarray(v) for k, v in params["mem"].items()}, "pm", pp)
    _prep_mem({k: np.asarray(v) for k, v in gp["mem"].items()}, "gm", pp)
    _prep_mem({k: np.asarray(v) for k, v in gp["resmem"].items()}, "gr", pp)

    # per-head channel regrouping: new channel d*64+h <- old channel h*4+d
    perm = np.array([h * HEAD + d for d in range(HEAD) for h in range(H)])
    q_w = np.asarray(gp["q_w"], f)[perm]
    k_w = np.asarray(gp["k_w"], f)[perm]
    v_w = np.asarray(gp["v_w"], f)[perm]
    q_b = np.asarray(gp["q_b"], f)[perm]
    k_b = np.asarray(gp["k_b"], f)[perm]
    v_b = np.asarray(gp["v_b"], f)[perm]
    pp["q_wT"] = np.ascontiguousarray(q_w.T)  # [64, 256]
    pp["k_wT"] = np.ascontiguousarray(k_w.T)
    pp["qb2"] = np.ascontiguousarray(q_b.reshape(2, 128).T)  # [128, 2]
    pp["kb2"] = np.ascontiguousarray(k_b.reshape(2, 128).T)
    # v in node-major layout with per-head bias row and ones column:
    # lhsT will be [rmem; ones] so row 64 of v_rhs supplies bias, col 64 of
    # each head block supplies the softmax denominator.
    v_rhs = np.zeros((65, HEAD * 65), f)
    for d in range(HEAD):
        v_rhs[0:64, d * 65 : d * 65 + 64] = v_w[d * 64 : (d + 1) * 64].T
        v_rhs[64, d * 65 : d * 65 + 64] = v_b[d * 64 : (d + 1) * 64]
        v_rhs[64, d * 65 + 64] = 1.0
    pp["v_rhs"] = v_rhs

    stru = np.asarray(gp["stru"], f)
    pp["Wt"] = np.ascontiguousarray(
        (np.clip(stru / 6.0 + 0.5, 0.0, 1.0) / np.sqrt(f(H))).T
    )  # [307, 307], indexed [key m, query n]

    d1_wT = np.ascontiguousarray(np.asarray(gp["d1_w"], f).T)  # [256, 64]
    pp["d1_wT0"] = d1_wT[0:128].copy()
    pp["d1_wT1"] = d1_wT[128:256].copy()
    pp["d1_b"] = np.asarray(gp["d1_b"], f).reshape(H, 1)

    for nm in ("period", "residual"):
        bp = params[nm]
        c1 = np.asarray(bp["c1_w"], f)  # [P, 64, 64]
        c2 = np.asarray(bp["c2_w"], f)
        # lhsT for block i is c1[i].T; store as [64(k), P, 64(m)]
        pp[nm + "_c1T"] = np.ascontiguousarray(np.transpose(c1, (2, 0, 1)))
        pp[nm + "_c2T"] = np.ascontiguousarray(np.transpose(c2, (2, 0, 1)))
        pp[nm + "_c1b"] = np.ascontiguousarray(np.asarray(bp["c1_b"], f).T)  # [64, P]
        pp[nm + "_c2b"] = np.ascontiguousarray(np.asarray(bp["c2_b"], f).T)
    pp["pred_wT"] = np.ascontiguousarray(np.asarray(params["pred_w"], f).T)  # [64, 12]
    pp["pred_b"] = np.asarray(params["pred_b"], f).reshape(OUT, 1)
    return pp


_PARAM_SHAPES = {
    "time_emb": (288, H), "day_emb": (7, H),
    "td_lw_r": (128, H), "fused_w": (12, T, H), "r_node": (H, N), "l1_bt": (H, T),
    "td_lw_p": (128, H), "p_w_x": (12, H), "p_node": (H, N),
    "q_wT": (H, 256), "k_wT": (H, 256), "qb2": (128, 2), "kb2": (128, 2),
    "v_rhs": (65, HEAD * 65), "Wt": (N, N),
    "d1_wT0": (128, H), "d1_wT1": (128, H), "d1_b": (H, 1),
    "period_c1T": (H, P, H), "period_c2T": (H, P, H),
    "period_c1b": (H, P), "period_c2b": (H, P),
    "residual_c1T": (H, P, H), "residual_c2T": (H, P, H),
    "residual_c1b": (H, P), "residual_c2b": (H, P),
    "pred_wT": (H, OUT), "pred_b": (OUT, 1),
}
for _pfx in ("pm", "gm", "gr"):
    _PARAM_SHAPES.update({
        _pfx + "_c1T": (H, H), _pfx + "_c1b": (H, 1), _pfx + "_msT": (H, M),
        _pfx + "_negent": (M, 1), _pfx + "_m1": (M, H), _pfx + "_c2T": (2 * H, H),
        _pfx + "_c2b": (H, 1),
    })


# --------------------------------------------------------------------------
# device program
# --------------------------------------------------------------------------

def _build_program(nc):
    dram = {}
    for name, shape in _PARAM_SHAPES.items():
        dram[name] = nc.dram_tensor(name, list(shape), F32, kind="ExternalInput").ap()
    xb_d = nc.dram_tensor("xb", [BPC, T, N], F32, kind="ExternalInput").ap()
    tdid_d = nc.dram_tensor("tdid", [BPC, 2, N], I32, kind="ExternalInput").ap()
    out_d = nc.dram_tensor("out", [BPC, OUT, N], F32, kind="ExternalOutput").ap()

    with ExitStack() as ctx:
        tc = ctx.enter_context(tile.TileContext(nc))
        const = ctx.enter_context(tc.tile_pool(name="const", bufs=1))
        pb = ctx.enter_context(tc.tile_pool(name="perb", bufs=2))
        wk = ctx.enter_context(tc.tile_pool(name="wk", bufs=3))
        ps_mm = ctx.enter_context(tc.tile_pool(name="psmm", bufs=3, space="PSUM"))
        ps_tr = ctx.enter_context(tc.tile_pool(name="pstr", bufs=2, space="PSUM"))
        ps_o = ctx.enter_context(tc.tile_pool(name="pso", bufs=2, space="PSUM"))

        ident = const.tile([128, 128], F32, tag="ident", name="ident")
        make_identity(nc, ident[:, :])

        cs_t = {}
        for name, shape in _PARAM_SHAPES.items():
            if name in ("time_emb", "day_emb"):
                continue  # gather tables stay in DRAM
            if name == "Wt":
                continue
            cs_t[name] = const.tile(list(shape), F32, tag=name, name="c_" + name)
            nc.sync.dma_start(cs_t[name][:], dram[name][:])
        wt_sb = []
        for ci, (c0, c1) in enumerate(CH):
            t_ = const.tile([128, N], F32, tag=f"wt{ci}", name=f"wt{ci}")
            nc.sync.dma_start(t_[0 : c1 - c0, :], dram["Wt"][c0:c1, :])
            wt_sb.append(t_)

        def mem_module(pfx, inp_ap, residual, out_tile, orow):
            """inp_ap: SBUF [64, N]; writes out rows [orow:orow+64] of out_tile."""
            pe = ps_mm.tile([H, N], F32, tag="mm", name="pe")
            nc.tensor.matmul(out=pe[:, :], lhsT=cs_t[pfx + "_c1T"][:, :],
                             rhs=inp_ap, start=True, stop=True)
            stack = wk.tile([128, N], F32, tag="stack", name="stack")
            nc.scalar.activation(stack[0:H, :], pe[:, :], AF.Relu,
                                 bias=cs_t[pfx + "_c1b"][:, 0:1])
            pA = ps_mm.tile([M, N], F32, tag="mm", name="pA")
            nc.tensor.matmul(out=pA[:, :], lhsT=cs_t[pfx + "_msT"][:, :],
                             rhs=stack[0:H, :], start=True, stop=True)
            A_sb = wk.tile([M, N], F32, tag="A_sb", name="A_sb")
            nc.scalar.activation(A_sb[:, :], pA[:, :], AF.Identity,
                                 bias=cs_t[pfx + "_negent"][:, 0:1])
            ohT = wk.tile([M, N], F32, tag="ohT", name="ohT")
            for (c0, c1) in CH:
                cnt = c1 - c0
                pAt = ps_tr.tile([128, M], F32, tag="tr", name="pAt")
                nc.tensor.transpose(pAt[0:cnt, :], A_sb[:, c0:c1], ident[0:M, 0:M])
                mx = wk.tile([128, 1], F32, tag="mx", name="mx")
                nc.vector.tensor_reduce(mx[0:cnt, 0:1], pAt[0:cnt, :],
                                        axis=AX.X, op=ALU.max)
                mask = wk.tile([128, M], F32, tag="mask", name="mask")
                nc.vector.tensor_scalar(mask[0:cnt, :], pAt[0:cnt, :],
                                        mx[0:cnt, 0:1], None, op0=ALU.is_ge)
                cs = wk.tile([128, M], F32, tag="cs", name="cs")
                nc.vector.tensor_tensor_scan(cs[0:cnt, :], mask[0:cnt, :],
                                             mask[0:cnt, :], 0.0,
                                             op0=ALU.add, op1=ALU.bypass)
                eq = wk.tile([128, M], F32, tag="eq", name="eq")
                nc.vector.tensor_scalar(eq[0:cnt, :], cs[0:cnt, :], 1.0, None,
                                        op0=ALU.is_equal)
                oh = wk.tile([128, M], F32, tag="oh", name="oh")
                nc.vector.tensor_tensor(oh[0:cnt, :], eq[0:cnt, :], mask[0:cnt, :],
                                        op=ALU.mult)
                poh = ps_tr.tile([M, 128], F32, tag="tr", name="poh")
                nc.tensor.transpose(poh[:, 0:cnt], oh[0:cnt, :], ident[0:cnt, 0:cnt])
                nc.scalar.activation(ohT[:, c0:c1], poh[:, 0:cnt], AF.Copy)
            pg = ps_mm.tile([H, N], F32, tag="mm", name="pg")
            nc.tensor.matmul(out=pg[:, :], lhsT=cs_t[pfx + "_m1"][:, :],
                             rhs=ohT[:, :], start=True, stop=True)
            if residual:
                nc.vector.tensor_tensor(stack[H:128, :], stack[0:H, :], pg[:, :],
                                        op=ALU.subtract)
            else:
                nc.scalar.activation(stack[H:128, :], pg[:, :], AF.Copy)
            po = ps_mm.tile([H, N], F32, tag="mm", name="po")
            nc.tensor.matmul(out=po[:, :], lhsT=cs_t[pfx + "_c2T"][:, :],
                             rhs=stack[:, :], start=True, stop=True)
            nc.scalar.activation(out_tile[orow : orow + H, :], po[:, :], AF.Identity,
                                 bias=cs_t[pfx + "_c2b"][:, 0:1])

        def res_block(nm, i, x_ap):
            ph = ps_mm.tile([H, N], F32, tag="mm", name="ph")
            nc.tensor.matmul(out=ph[:, :], lhsT=cs_t[nm + "_c1T"][:, i, :], rhs=x_ap,
                             start=True, stop=True)
            h_sb = wk.tile([H, N], F32, tag="rbh", name="h_sb")
            nc.scalar.activation(h_sb[:, :], ph[:, :], AF.Relu,
                                 bias=cs_t[nm + "_c1b"][:, i : i + 1])
            p2 = ps_mm.tile([H, N], F32, tag="mm", name="p2")
            nc.tensor.matmul(out=p2[:, :], lhsT=cs_t[nm + "_c2T"][:, i, :],
                             rhs=h_sb[:, :], start=True, stop=True)
            t2 = wk.tile([H, N], F32, tag="rbt", name="t2")
            nc.scalar.activation(t2[:, :], p2[:, :], AF.Identity,
                                 bias=cs_t[nm + "_c2b"][:, i : i + 1])
            nc.vector.tensor_tensor(x_ap, x_ap, t2[:, :], op=ALU.add)

        for b in range(BPC):
            # ---- input slices ----
            xb_sb = pb.tile([T, N], F32, tag="xb", name="xb_sb")
            nc.sync.dma_start(xb_sb[:, :], xb_d[b])

            # ---- time/day embedding gathers -> td_sb [128, N] ----
            td_sb = pb.tile([128, N], F32, tag="td", name="td_sb")
            for row0, tbl, trow in ((0, "time_emb", 0), (H, "day_emb", 1)):
                for (c0, c1) in CH:
                    cnt = c1 - c0
                    idx = wk.tile([128, 1], I32, tag="idx", name="idx")
                    nc.sync.dma_start(idx[0:cnt, 0:1], tdid_d[b, trow, c0:c1])
                    g = wk.tile([128, H], F32, tag="gath", name="g")
                    nc.gpsimd.indirect_dma_start(
                        out=g[0:cnt, :], out_offset=None, in_=dram[tbl][:, :],
                        in_offset=bass.IndirectOffsetOnAxis(ap=idx[0:cnt, 0:1], axis=0),
                    )
                    pt = ps_tr.tile([H, 128], F32, tag="tr", name="pt")
                    nc.tensor.transpose(pt[:, 0:cnt], g[0:cnt, :], ident[0:cnt, 0:cnt])
                    nc.scalar.activation(td_sb[row0 : row0 + H, c0:c1], pt[:, 0:cnt],
                                         AF.Copy)

            # ---- p path ----
            pp_ps = ps_mm.tile([H, N], F32, tag="mm", name="pp_ps")
            nc.tensor.matmul(out=pp_ps[:, :], lhsT=cs_t["td_lw_p"][:, :],
                             rhs=td_sb[:, :], start=True, stop=False)
            nc.tensor.matmul(out=pp_ps[:, :], lhsT=cs_t["p_w_x"][:, :],
                             rhs=xb_sb[:, :], start=False, stop=True)
            p_sb = wk.tile([H, N], F32, tag="p0", name="p_sb")
            nc.vector.tensor_tensor(p_sb[:, :], pp_ps[:, :], cs_t["p_node"][:, :],
                                    op=ALU.add)
            pout = pb.tile([H, N], F32, tag="pout", name="pout")
            mem_module("pm", p_sb[:, :], False, pout, 0)
            for i in range(P):
                res_block("period", i, pout[:, :])

            # ---- r base (shared across t) ----
            prb = ps_mm.tile([H, N], F32, tag="mm", name="prb")
            nc.tensor.matmul(out=prb[:, :], lhsT=cs_t["td_lw_r"][:, :],
                             rhs=td_sb[:, :], start=True, stop=True)
            rbase = pb.tile([H, N], F32, tag="rbase", name="rbase")
            nc.vector.tensor_tensor(rbase[:, :], prb[:, :], cs_t["r_node"][:, :],
                                    op=ALU.add)

            # ---- attention accumulator [n, (d,chunk,h)] ----
            oacc = pb.tile([128, HEAD * 3 * H], F32, tag="oacc", name="oacc")
            nc.gpsimd.memset(oacc[:, :], 0.0)

            for t in range(T):
                prt = ps_mm.tile([H, N], F32, tag="mm", name="prt")
                nc.tensor.matmul(out=prt[:, :], lhsT=cs_t["fused_w"][:, t, :],
                                 rhs=xb_sb[:, :], start=True, stop=True)
                r_sb = wk.tile([H, N], F32, tag="r_sb", name="r_sb")
                nc.scalar.activation(r_sb[:, :], prt[:, :], AF.Identity,
                                     bias=cs_t["l1_bt"][:, t : t + 1])
                nc.vector.tensor_tensor(r_sb[:, :], r_sb[:, :], rbase[:, :],
                                        op=ALU.add)

                mem_sb = wk.tile([H, N], F32, tag="mem", name="mem_sb")
                mem_module("gm", r_sb[:, :], False, mem_sb, 0)
                rmem_sb = wk.tile([65, N], F32, tag="rmem", name="rmem_sb")
                mem_module("gr", r_sb[:, :], True, rmem_sb, 0)
                nc.gpsimd.memset(rmem_sb[64:65, :], 1.0)

                qk_t = {}
                for nm, wT, b2 in (("q", "q_wT", "qb2"), ("k", "k_wT", "kb2")):
                    for c in range(2):
                        pq = ps_mm.tile([128, N], F32, tag="mm", name="pq")
                        nc.tensor.matmul(out=pq[:, :],
                                         lhsT=cs_t[wT][:, c * 128 : (c + 1) * 128],
                                         rhs=mem_sb[:, :], start=True, stop=True)
                        t_sb = wk.tile([128, N], F32, tag=f"{nm}{c}", name="t_sb")
                        nc.scalar.activation(t_sb[:, :], pq[:, :], AF.Relu,
                                             bias=cs_t[b2][:, c : c + 1])
                        qk_t[(nm, c)] = t_sb

                v_t = []
                for ci, (c0, c1) in enumerate(CH):
                    cnt = c1 - c0
                    pv = ps_mm.tile([128, HEAD * 65], F32, tag="mm", name="pv")
                    nc.tensor.matmul(out=pv[0:cnt, :], lhsT=rmem_sb[:, c0:c1],
                                     rhs=cs_t["v_rhs"][:, :], start=True, stop=True)
                    vt = wk.tile([128, HEAD * 65], F32, tag=f"v{ci}", name="vt")
                    nc.scalar.activation(vt[0:cnt, :], pv[0:cnt, :], AF.Relu)
                    v_t.append(vt)

                for d in range(HEAD):
                    qt = qk_t[("q", d // 2)]
                    kt = qk_t[("k", d // 2)]
                    rr = (d % 2) * H
                    po_ = ps_o.tile([65, N], F32, tag="po", name="po_")
                    for ci, (c0, c1) in enumerate(CH):
                        cnt = c1 - c0
                        pS = ps_mm.tile([128, N], F32, tag="mm", name="pS")
                        nc.tensor.matmul(out=pS[0:cnt, :],
                                         lhsT=kt[rr : rr + H, c0:c1],
                                         rhs=qt[rr : rr + H, :],
                                         start=True, stop=True)
                        es = wk.tile([128, N], F32, tag="es", name="es")
                        nc.vector.tensor_tensor(es[0:cnt, :], pS[0:cnt, :],
                                                wt_sb[ci][0:cnt, :], op=ALU.mult)
                        ee = wk.tile([128, N], F32, tag="ee", name="ee")
                        nc.scalar.activation(ee[0:cnt, :], es[0:cnt, :], AF.Exp)
                        nc.tensor.matmul(out=po_[:, :],
                                         lhsT=v_t[ci][0:cnt, d * 65 : (d + 1) * 65],
                                         rhs=ee[0:cnt, :],
                                         start=(ci == 0), stop=(ci == 2))
                    oa_sb = wk.tile([65, N], F32, tag="oa", name="oa_sb")
                    nc.scalar.activation(oa_sb[:, :], po_[:, :], AF.Copy)
                    for ci, (c0, c1) in enumerate(CH):
                        cnt = c1 - c0
                        pot = ps_tr.tile([128, 65], F32, tag="tr", name="pot")
                        nc.tensor.transpose(pot[0:cnt, :], oa_sb[:, c0:c1],
                                            ident[0:65, 0:65])
                        rec = wk.tile([128, 1], F32, tag="rec", name="rec")
                        nc.vector.reciprocal(rec[0:cnt, 0:1], pot[0:cnt, 64:65])
                        tmp = wk.tile([128, H], F32, tag="otmp", name="tmp")
                        nc.scalar.activation(tmp[0:cnt, :], pot[0:cnt, 0:H], AF.Copy,
                                             scale=rec[0:cnt, 0:1])
                        col = (d * 3 + ci) * H
                        nc.vector.tensor_tensor(oacc[0:cnt, col : col + H],
                                                oacc[0:cnt, col : col + H],
                                                tmp[0:cnt, :], op=ALU.add)

            # ---- o_flat^T assembly: two [128, N] tiles (heads 01 / 23) ----
            of = []
            for half in range(2):
                of_sb = pb.tile([128, N], F32, tag=f"of{half}", name="of_sb")
                of.append(of_sb)
            for d in range(HEAD):
                for ci, (c0, c1) in enumerate(CH):
                    cnt = c1 - c0
                    col = (d * 3 + ci) * H
                    pob = ps_tr.tile([H, 128], F32, tag="tr", name="pob")
                    nc.tensor.transpose(pob[:, 0:cnt], oacc[0:cnt, col : col + H],
                                        ident[0:cnt, 0:cnt])
                    nc.scalar.activation(
                        of[d // 2][(d % 2) * H : (d % 2) * H + H, c0:c1],
                        pob[:, 0:cnt], AF.Copy)

            pd = ps_mm.tile([H, N], F32, tag="mm", name="pd")
            nc.tensor.matmul(out=pd[:, :], lhsT=cs_t["d1_wT0"][:, :], rhs=of[0][:, :],
                             start=True, stop=False)
            nc.tensor.matmul(out=pd[:, :], lhsT=cs_t["d1_wT1"][:, :], rhs=of[1][:, :],
                             start=False, stop=True)
            rr_sb = wk.tile([H, N], F32, tag="rr", name="rr_sb")
            nc.scalar.activation(rr_sb[:, :], pd[:, :], AF.Relu,
                                 bias=cs_t["d1_b"][:, 0:1])
            for i in range(P):
                res_block("residual", i, rr_sb[:, :])

            nc.vector.tensor_tensor(rr_sb[:, :], rr_sb[:, :], pout[:, :], op=ALU.add)
            ppr = ps_mm.tile([OUT, N], F32, tag="mm", name="ppr")
            nc.tensor.matmul(out=ppr[:, :], lhsT=cs_t["pred_wT"][:, :],
                             rhs=rr_sb[:, :], start=True, stop=True)
            outp = wk.tile([OUT, N], F32, tag="outp", name="outp")
            nc.scalar.activation(outp[:, :], ppr[:, :], AF.Identity,
                                 bias=cs_t["pred_b"][:, 0:1])
            nc.sync.dma_start(out_d[b], outp[:, :])
    return nc


_CACHED = {}


def _get_nc():
    if "nc" not in _CACHED:
        nc = bacc.Bacc("TRN2", target_bir_lowering=False, debug=False,
                       enable_asserts=False, num_devices=NCORES)
        _build_program(nc)
        nc.compile()
        _CACHED["nc"] = nc
    return _CACHED["nc"]


def _make_in_maps(x_data, time_id, day_id, params):
    x_data = np.asarray(x_data, np.float32)
    time_id = np.asarray(time_id, np.int32)
    day_id = np.asarray(day_id, np.int32)
    pp = _prep_params(params)
    in_maps = []
    for c in range(NCORES):
        b0, b1 = c * BPC, (c + 1) * BPC
        m = dict(pp)
        m["xb"] = np.ascontiguousarray(x_data[b0:b1])
        m["tdid"] = np.ascontiguousarray(
            np.stack([time_id[b0:b1, -1, :], day_id[b0:b1, -1, :]], 1)
        )
        in_maps.append(m)
    return in_maps


def kernel(x_data, time_id, day_id, params):
    in_maps = _make_in_maps(x_data, time_id, day_id, params)
    nc = _get_nc()
    res = run_bass_kernel_spmd(nc, in_maps, list(range(NCORES)))
    return np.concatenate([res.results[c]["out"] for c in range(NCORES)], 0)


# revision 13
# speedup vs baseline: 61.9148x; 61.9148x over previous
"""STPAN (spatio-temporal attention net) Trainium2 kernel.

Strategy: pure data-parallel over batch (B=16 -> 2 per core on 8 cores).
Everything for one batch element fits in SBUF, so the whole network runs
fused on-chip per core with no collectives; only the x_data/id slices go in
and the [2, 12, 307] output comes out.

Host-side precomputation restructures the algebra (all exact, fp32-safe):
  * embedding: the time/day/node/pos components of the big concat-matmuls
    are folded into per-batch and per-t biases / small fused weights
    (low-rank through T=12), eliminating the 5H and 4H concat GEMMs.
  * memory module: argmin_m KL(q_n || p_m) == argmax_m (ms @ emb - ent)[m]
    because the log-sum-exp term is constant in m and sum_h ms[m,h] = 1.
    No exp/log needed on device; ties broken first-index via a cumsum trick.
  * attention: qk values are tiny (<0.5 scaled) so softmax needs no
    max-subtraction; exp'd scores are consumed in [key, query] layout and
    the denominator is produced by an extra all-ones column appended to V.
"""

import sys

sys.path.insert(0, "/opt/trn_rl_repo")
sys.path.insert(0, "/opt/trn_rl_repo/concourse")

from contextlib import ExitStack

import numpy as np

import concourse.bass as bass
import concourse.tile as tile
from concourse import bacc, mybir
from concourse.bass_utils import run_bass_kernel_spmd
from concourse.masks import make_identity

F32 = mybir.dt.float32
I32 = mybir.dt.int32
AF = mybir.ActivationFunctionType
ALU = mybir.AluOpType
AX = mybir.AxisListType

B, T, N, H, HEAD, M, P, OUT = 16, 12, 307, 64, 4, 20, 2, 12
NCORES = 8
BPC = B // NCORES  # batches per core
CH = [(0, 128), (128, 256), (256, 307)]  # node-dim chunks


# --------------------------------------------------------------------------
# host-side parameter restructuring
# --------------------------------------------------------------------------

def _prep_mem(mp, pfx, out):
    m1 = np.asarray(mp["m1"], np.float64)
    e = np.exp(m1 - m1.max(-1, keepdims=True))
    ms = e / e.sum(-1, keepdims=True)
    logms = m1 - m1.max(-1, keepdims=True) - np.log(e.sum(-1, keepdims=True))
    ent = (ms * logms).sum(-1)
    out[pfx + "_c1T"] = np.ascontiguousarray(np.asarray(mp["c1_w"], np.float32).T)
    out[pfx + "_c1b"] = np.asarray(mp["c1_b"], np.float32).reshape(H, 1)
    out[pfx + "_msT"] = np.ascontiguousarray(ms.astype(np.float32).T)  # [64, 20]
    out[pfx + "_negent"] = (-ent).astype(np.float32).reshape(M, 1)
    out[pfx + "_m1"] = np.asarray(mp["m1"], np.float32)  # [20, 64] lhsT
    out[pfx + "_c2T"] = np.ascontiguousarray(np.asarray(mp["c2_w"], np.float32).T)  # [128, 64]
    out[pfx + "_c2b"] = np.asarray(mp["c2_b"], np.float32).reshape(H, 1)


def _prep_params(params):
    f = np.float32
    pp = {}
    ep = {k: np.asarray(v, f) for k, v in params["emb"].items()}
    gp = params["gma"]

    l1_w = ep["l1_w"]  # [64, 320]
    pp["time_emb"] = ep["time_emb"]  # [288, 64]
    pp["day_emb"] = ep["day_emb"]    # [7, 64]
    pp["td_lw_r"] = np.ascontiguousarray(l1_w[:, 0:128].T)  # [128, 64]
    # fused per-t weight: l1_w_ier @ dense_w[t] -> lhsT [12, 64]
    dense_w = ep["dense_w"]  # [T, H, T]
    pp["fused_w"] = np.stack(
        [np.ascontiguousarray((l1_w[:, 128:192] @ dense_w[t]).T) for t in range(T)],
        axis=1,
    ).astype(f)  # [12(k), T, 64(m)]
    pp["r_node"] = (l1_w[:, 192:256] @ ep["node_emb"].T).astype(f)  # [64, 307]
    pp["l1_bt"] = np.ascontiguousarray(
        (
            ep["l1_b"][None, :]
            + ep["pos_emb"] @ l1_w[:, 256:320].T
            + ep["dense_b"] @ l1_w[:, 128:192].T
        ).T
    ).astype(f)  # [64, 12]

    c2_w = ep["c2_w"]  # [64, 256]
    pp["td_lw_p"] = np.ascontiguousarray(c2_w[:, 0:128].T)  # [128, 64]
    pp["p_w_x"] = np.ascontiguousarray((c2_w[:, 128:192] @ ep["c1_w"]).T)  # [12, 64]
    pp["p_node"] = (
        c2_w[:, 192:256] @ ep["node_emb"].T
        + (ep["c2_b"] + c2_w[:, 128:192] @ ep["c1_b"])[:, None]
    ).astype(f)  # [64, 307]

    _prep_mem({k: np.asarray(v) for k, v in params["mem"].items()}, "pm", pp)
    _prep_mem({k: np.asarray(v) for k, v in gp["mem"].items()}, "gm", pp)
    _prep_mem({k: np.asarray(v) for k, v in gp["resmem"].items()}, "gr", pp)

    # per-head channel regrouping: new channel d*64+h <- old channel h*4+d
    perm = np.array([h * HEAD + d for d in range(HEAD) for h in range(H)])
    q_w = np.asarray(gp["q_w"], f)[perm]
    k_w = np.asarray(gp["k_w"], f)[perm]
    v_w = np.asarray(gp["v_w"], f)[perm]
    q_b = np.asarray(gp["q_b"], f)[perm]
    k_b = np.asarray(gp["k_b"], f)[perm]
    v_b = np.asarray(gp["v_b"], f)[perm]
    pp["q_wT"] = np.ascontiguousarray(q_w.T)  # [64, 256]
    pp["k_wT"] = np.ascontiguousarray(k_w.T)
    pp["qb2"] = np.ascontiguousarray(q_b.reshape(2, 128).T)  # [128, 2]
    pp["kb2"] = np.ascontiguousarray(k_b.reshape(2, 128).T)
    # v in node-major layout with per-head bias row and ones column:
    # lhsT will be [rmem; ones] so row 64 of v_rhs supplies bias, col 64 of
    # each head block supplies the softmax denominator.
    v_rhs = np.zeros((65, HEAD * 65), f)
    for d in range(HEAD):
        v_rhs[0:64, d * 65 : d * 65 + 64] = v_w[d * 64 : (d + 1) * 64].T
        v_rhs[64, d * 65 : d * 65 + 64] = v_b[d * 64 : (d + 1) * 64]
        v_rhs[64, d * 65 + 64] = 1.0
    pp["v_rhs"] = v_rhs

    stru = np.asarray(gp["stru"], f)
    pp["Wt"] = np.ascontiguousarray(
        (np.clip(stru / 6.0 + 0.5, 0.0, 1.0) / np.sqrt(f(H))).T
    )  # [307, 307], indexed [key m, query n]

    d1_wT = np.ascontiguousarray(np.asarray(gp["d1_w"], f).T)  # [256, 64]
    pp["d1_wT0"] = d1_wT[0:128].copy()
    pp["d1_wT1"] = d1_wT[128:256].copy()
    pp["d1_b"] = np.asarray(gp["d1_b"], f).reshape(H, 1)

    for nm in ("period", "residual"):
        bp = params[nm]
        c1 = np.asarray(bp["c1_w"], f)  # [P, 64, 64]
        c2 = np.asarray(bp["c2_w"], f)
        # lhsT for block i is c1[i].T; store as [64(k), P, 64(m)]
        pp[nm + "_c1T"] = np.ascontiguousarray(np.transpose(c1, (2, 0, 1)))
        pp[nm + "_c2T"] = np.ascontiguousarray(np.transpose(c2, (2, 0, 1)))
        pp[nm + "_c1b"] = np.ascontiguousarray(np.asarray(bp["c1_b"], f).T)  # [64, P]
        pp[nm + "_c2b"] = np.ascontiguousarray(np.asarray(bp["c2_b"], f).T)
    pp["pred_wT"] = np.ascontiguousarray(np.asarray(params["pred_w"], f).T)  # [64, 12]
    pp["pred_b"] = np.asarray(params["pred_b"], f).reshape(OUT, 1)
    return pp


_PARAM_SHAPES = {
    "time_emb": (288, H), "day_emb": (7, H),
    "td_lw_r": (128, H), "fused_w": (12, T, H), "r_node": (H, N), "l1_bt": (H, T),
    "td_lw_p": (128, H), "p_w_x": (12, H), "p_node": (H, N),
    "q_wT": (H, 256), "k_wT": (H, 256), "qb2": (128, 2), "kb2": (128, 2),
    "v_rhs": (65, HEAD * 65), "Wt": (N, N),
    "d1_wT0": (128, H), "d1_wT1": (128, H), "d1_b": (H, 1),
    "period_c1T": (H, P, H), "period_c2T": (H, P, H),
    "period_c1b": (H, P), "period_c2b": (H, P),
    "residual_c1T": (H, P, H), "residual_c2T": (H, P, H),
    "residual_c1b": (H, P), "residual_c2b": (H, P),
    "pred_wT": (H, OUT), "pred_b": (OUT, 1),
}
for _pfx in ("pm", "gm", "gr"):
    _PARAM_SHAPES.update({
        _pfx + "_c1T": (H, H), _pfx + "_c1b": (H, 1), _pfx + "_msT": (H, M),
        _pfx + "_negent": (M, 1), _pfx + "_m1": (M, H), _pfx + "_c2T": (2 * H, H),
        _pfx + "_c2b": (H, 1),
    })


# --------------------------------------------------------------------------
# device program
# --------------------------------------------------------------------------

def _build_program(nc, reps=1):
    dram = {}
    for name, shape in _PARAM_SHAPES.items():
        dram[name] = nc.dram_tensor(name, list(shape), F32, kind="ExternalInput").ap()
    xb_d = nc.dram_tensor("xb", [BPC, T, N], F32, kind="ExternalInput").ap()
    tdid_d = nc.dram_tensor("tdid", [BPC, 2, N], I32, kind="ExternalInput").ap()
    out_d = nc.dram_tensor("out", [BPC, OUT, N], F32, kind="ExternalOutput").ap()

    with ExitStack() as ctx:
        tc = ctx.enter_context(tile.TileContext(nc))
        const = ctx.enter_context(tc.tile_pool(name="const", bufs=1))
        pb = ctx.enter_context(tc.tile_pool(name="perb", bufs=2))
        wk = ctx.enter_context(tc.tile_pool(name="wk", bufs=3))
        ps_mm = ctx.enter_context(tc.tile_pool(name="psmm", bufs=3, space="PSUM"))
        ps_tr = ctx.enter_context(tc.tile_pool(name="pstr", bufs=2, space="PSUM"))
        ps_o = ctx.enter_context(tc.tile_pool(name="pso", bufs=2, space="PSUM"))

        ident = const.tile([128, 128], F32, tag="ident", name="ident")
        make_identity(nc, ident[:, :])

        cs_t = {}
        for name, shape in _PARAM_SHAPES.items():
            if name in ("time_emb", "day_emb"):
                continue  # gather tables stay in DRAM
            if name == "Wt":
                continue
            cs_t[name] = const.tile(list(shape), F32, tag=name, name="c_" + name)
            nc.sync.dma_start(cs_t[name][:], dram[name][:])
        wt_sb = []
        for ci, (c0, c1) in enumerate(CH):
            t_ = const.tile([128, N], F32, tag=f"wt{ci}", name=f"wt{ci}")
            nc.sync.dma_start(t_[0 : c1 - c0, :], dram["Wt"][c0:c1, :])
            wt_sb.append(t_)

        rep_loop = tc.For_i(0, reps, 1) if reps > 1 else None
        if rep_loop is not None:
            ctx.enter_context(rep_loop)

        def mem_module(pfx, inp_ap, residual, out_tile, orow):
            """inp_ap: SBUF [64, N]; writes out rows [orow:orow+64] of out_tile."""
            pe = ps_mm.tile([H, N], F32, tag="mm", name="pe")
            nc.tensor.matmul(out=pe[:, :], lhsT=cs_t[pfx + "_c1T"][:, :],
                             rhs=inp_ap, start=True, stop=True)
            stack = wk.tile([128, N], F32, tag="stack", name="stack")
            nc.scalar.activation(stack[0:H, :], pe[:, :], AF.Relu,
                                 bias=cs_t[pfx + "_c1b"][:, 0:1])
            pA = ps_mm.tile([M, N], F32, tag="mm", name="pA")
            nc.tensor.matmul(out=pA[:, :], lhsT=cs_t[pfx + "_msT"][:, :],
                             rhs=stack[0:H, :], start=True, stop=True)
            A_sb = wk.tile([M, N], F32, tag="A_sb", name="A_sb")
            nc.scalar.activation(A_sb[:, :], pA[:, :], AF.Identity,
                                 bias=cs_t[pfx + "_negent"][:, 0:1])
            ohT = wk.tile([M, N], F32, tag="ohT", name="ohT")
            for (c0, c1) in CH:
                cnt = c1 - c0
                pAt = ps_tr.tile([128, M], F32, tag="tr", name="pAt")
                nc.tensor.transpose(pAt[0:cnt, :], A_sb[:, c0:c1], ident[0:M, 0:M])
                mx = wk.tile([128, 1], F32, tag="mx", name="mx")
                nc.vector.tensor_reduce(mx[0:cnt, 0:1], pAt[0:cnt, :],
                                        axis=AX.X, op=ALU.max)
                mask = wk.tile([128, M], F32, tag="mask", name="mask")
                nc.vector.tensor_scalar(mask[0:cnt, :], pAt[0:cnt, :],
                                        mx[0:cnt, 0:1], None, op0=ALU.is_ge)
                cs = wk.tile([128, M], F32, tag="cs", name="cs")
                nc.vector.tensor_tensor_scan(cs[0:cnt, :], mask[0:cnt, :],
                                             mask[0:cnt, :], 0.0,
                                             op0=ALU.add, op1=ALU.bypass)
                eq = wk.tile([128, M], F32, tag="eq", name="eq")
                nc.vector.tensor_scalar(eq[0:cnt, :], cs[0:cnt, :], 1.0, None,
                                        op0=ALU.is_equal)
                oh = wk.tile([128, M], F32, tag="oh", name="oh")
                nc.vector.tensor_tensor(oh[0:cnt, :], eq[0:cnt, :], mask[0:cnt, :],
                                        op=ALU.mult)
                poh = ps_tr.tile([M, 128], F32, tag="tr", name="poh")
                nc.tensor.transpose(poh[:, 0:cnt], oh[0:cnt, :], ident[0:cnt, 0:cnt])
                nc.scalar.activation(ohT[:, c0:c1], poh[:, 0:cnt], AF.Copy)
            pg = ps_mm.tile([H, N], F32, tag="mm", name="pg")
            nc.tensor.matmul(out=pg[:, :], lhsT=cs_t[pfx + "_m1"][:, :],
                             rhs=ohT[:, :], start=True, stop=True)
            if residual:
                nc.vector.tensor_tensor(stack[H:128, :], stack[0:H, :], pg[:, :],
                                        op=ALU.subtract)
            else:
                nc.scalar.activation(stack[H:128, :], pg[:, :], AF.Copy)
            po = ps_mm.tile([H, N], F32, tag="mm", name="po")
            nc.tensor.matmul(out=po[:, :], lhsT=cs_t[pfx + "_c2T"][:, :],
                             rhs=stack[:, :], start=True, stop=True)
            nc.scalar.activation(out_tile[orow : orow + H, :], po[:, :], AF.Identity,
                                 bias=cs_t[pfx + "_c2b"][:, 0:1])

        def res_block(nm, i, x_ap):
            ph = ps_mm.tile([H, N], F32, tag="mm", name="ph")
            nc.tensor.matmul(out=ph[:, :], lhsT=cs_t[nm + "_c1T"][:, i, :], rhs=x_ap,
                             start=True, stop=True)
            h_sb = wk.tile([H, N], F32, tag="rbh", name="h_sb")
            nc.scalar.activation(h_sb[:, :], ph[:, :], AF.Relu,
                                 bias=cs_t[nm + "_c1b"][:, i : i + 1])
            p2 = ps_mm.tile([H, N], F32, tag="mm", name="p2")
            nc.tensor.matmul(out=p2[:, :], lhsT=cs_t[nm + "_c2T"][:, i, :],
                             rhs=h_sb[:, :], start=True, stop=True)
            t2 = wk.tile([H, N], F32, tag="rbt", name="t2")
            nc.scalar.activation(t2[:, :], p2[:, :], AF.Identity,
                                 bias=cs_t[nm + "_c2b"][:, i : i + 1])
            nc.vector.tensor_tensor(x_ap, x_ap, t2[:, :], op=ALU.add)

        for b in range(BPC):
            # ---- input slices ----
            xb_sb = pb.tile([T, N], F32, tag="xb", name="xb_sb")
            nc.sync.dma_start(xb_sb[:, :], xb_d[b])

            # ---- time/day embedding gathers -> td_sb [128, N] ----
            td_sb = pb.tile([128, N], F32, tag="td", name="td_sb")
            for row0, tbl, trow in ((0, "time_emb", 0), (H, "day_emb", 1)):
                for (c0, c1) in CH:
                    cnt = c1 - c0
                    idx = wk.tile([128, 1], I32, tag="idx", name="idx")
                    nc.sync.dma_start(idx[0:cnt, 0:1], tdid_d[b, trow, c0:c1])
                    g = wk.tile([128, H], F32, tag="gath", name="g")
                    nc.gpsimd.indirect_dma_start(
                        out=g[0:cnt, :], out_offset=None, in_=dram[tbl][:, :],
                        in_offset=bass.IndirectOffsetOnAxis(ap=idx[0:cnt, 0:1], axis=0),
                    )
                    pt = ps_tr.tile([H, 128], F32, tag="tr", name="pt")
                    nc.tensor.transpose(pt[:, 0:cnt], g[0:cnt, :], ident[0:cnt, 0:cnt])
                    nc.scalar.activation(td_sb[row0 : row0 + H, c0:c1], pt[:, 0:cnt],
                                         AF.Copy)

            # ---- p path ----
            pp_ps = ps_mm.tile([H, N], F32, tag="mm", name="pp_ps")
            nc.tensor.matmul(out=pp_ps[:, :], lhsT=cs_t["td_lw_p"][:, :],
                             rhs=td_sb[:, :], start=True, stop=False)
            nc.tensor.matmul(out=pp_ps[:, :], lhsT=cs_t["p_w_x"][:, :],
                             rhs=xb_sb[:, :], start=False, stop=True)
            p_sb = wk.tile([H, N], F32, tag="p0", name="p_sb")
            nc.vector.tensor_tensor(p_sb[:, :], pp_ps[:, :], cs_t["p_node"][:, :],
                                    op=ALU.add)
            pout = pb.tile([H, N], F32, tag="pout", name="pout")
            mem_module("pm", p_sb[:, :], False, pout, 0)
            for i in range(P):
                res_block("period", i, pout[:, :])

            # ---- r base (shared across t) ----
            prb = ps_mm.tile([H, N], F32, tag="mm", name="prb")
            nc.tensor.matmul(out=prb[:, :], lhsT=cs_t["td_lw_r"][:, :],
                             rhs=td_sb[:, :], start=True, stop=True)
            rbase = pb.tile([H, N], F32, tag="rbase", name="rbase")
            nc.vector.tensor_tensor(rbase[:, :], prb[:, :], cs_t["r_node"][:, :],
                                    op=ALU.add)

            # ---- attention accumulator [n, (d,chunk,h)] ----
            oacc = pb.tile([128, HEAD * 3 * H], F32, tag="oacc", name="oacc")
            nc.gpsimd.memset(oacc[:, :], 0.0)

            for t in range(T):
                prt = ps_mm.tile([H, N], F32, tag="mm", name="prt")
                nc.tensor.matmul(out=prt[:, :], lhsT=cs_t["fused_w"][:, t, :],
                                 rhs=xb_sb[:, :], start=True, stop=True)
                r_sb = wk.tile([H, N], F32, tag="r_sb", name="r_sb")
                nc.scalar.activation(r_sb[:, :], prt[:, :], AF.Identity,
                                     bias=cs_t["l1_bt"][:, t : t + 1])
                nc.vector.tensor_tensor(r_sb[:, :], r_sb[:, :], rbase[:, :],
                                        op=ALU.add)

                mem_sb = wk.tile([H, N], F32, tag="mem", name="mem_sb")
                mem_module("gm", r_sb[:, :], False, mem_sb, 0)
                rmem_sb = wk.tile([65, N], F32, tag="rmem", name="rmem_sb")
                mem_module("gr", r_sb[:, :], True, rmem_sb, 0)
                nc.gpsimd.memset(rmem_sb[64:65, :], 1.0)

                qk_t = {}
                for nm, wT, b2 in (("q", "q_wT", "qb2"), ("k", "k_wT", "kb2")):
                    for c in range(2):
                        pq = ps_mm.tile([128, N], F32, tag="mm", name="pq")
                        nc.tensor.matmul(out=pq[:, :],
                                         lhsT=cs_t[wT][:, c * 128 : (c + 1) * 128],
                                         rhs=mem_sb[:, :], start=True, stop=True)
                        t_sb = wk.tile([128, N], F32, tag=f"{nm}{c}", name="t_sb")
                        nc.scalar.activation(t_sb[:, :], pq[:, :], AF.Relu,
                                             bias=cs_t[b2][:, c : c + 1])
                        qk_t[(nm, c)] = t_sb

                v_t = []
                for ci, (c0, c1) in enumerate(CH):
                    cnt = c1 - c0
                    pv = ps_mm.tile([128, HEAD * 65], F32, tag="mm", name="pv")
                    nc.tensor.matmul(out=pv[0:cnt, :], lhsT=rmem_sb[:, c0:c1],
                                     rhs=cs_t["v_rhs"][:, :], start=True, stop=True)
                    vt = wk.tile([128, HEAD * 65], F32, tag=f"v{ci}", name="vt")
                    nc.scalar.activation(vt[0:cnt, :], pv[0:cnt, :], AF.Relu)
                    v_t.append(vt)

                for d in range(HEAD):
                    qt = qk_t[("q", d // 2)]
                    kt = qk_t[("k", d // 2)]
                    rr = (d % 2) * H
                    po_ = ps_o.tile([65, N], F32, tag="po", name="po_")
                    for ci, (c0, c1) in enumerate(CH):
                        cnt = c1 - c0
                        pS = ps_mm.tile([128, N], F32, tag="mm", name="pS")
                        nc.tensor.matmul(out=pS[0:cnt, :],
                                         lhsT=kt[rr : rr + H, c0:c1],
                                         rhs=qt[rr : rr + H, :],
                                         start=True, stop=True)
                        es = wk.tile([128, N], F32, tag="es", name="es")
                        nc.vector.tensor_tensor(es[0:cnt, :], pS[0:cnt, :],
                                                wt_sb[ci][0:cnt, :], op=ALU.mult)
                        ee = wk.tile([128, N], F32, tag="ee", name="ee")
                        nc.scalar.activation(ee[0:cnt, :], es[0:cnt, :], AF.Exp)
                        nc.tensor.matmul(out=po_[:, :],
                                         lhsT=v_t[ci][0:cnt, d * 65 : (d + 1) * 65],
                                         rhs=ee[0:cnt, :],
                                         start=(ci == 0), stop=(ci == 2))
                    oa_sb = wk.tile([65, N], F32, tag="oa", name="oa_sb")
                    nc.scalar.activation(oa_sb[:, :], po_[:, :], AF.Copy)
                    for ci, (c0, c1) in enumerate(CH):
                        cnt = c1 - c0
                        pot = ps_tr.tile([128, 65], F32, tag="tr", name="pot")
                        nc.tensor.transpose(pot[0:cnt, :], oa_sb[:, c0:c1],
                                            ident[0:65, 0:65])
                        rec = wk.tile([128, 1], F32, tag="rec", name="rec")
                        nc.vector.reciprocal(rec[0:cnt, 0:1], pot[0:cnt, 64:65])
                        tmp = wk.tile([128, H], F32, tag="otmp", name="tmp")
                        nc.scalar.activation(tmp[0:cnt, :], pot[0:cnt, 0:H], AF.Copy,
                                             scale=rec[0:cnt, 0:1])
                        col = (d * 3 + ci) * H
                        nc.vector.tensor_tensor(oacc[0:cnt, col : col + H],
                                                oacc[0:cnt, col : col + H],
                                                tmp[0:cnt, :], op=ALU.add)

            # ---- o_flat^T assembly: two [128, N] tiles (heads 01 / 23) ----
            of = []
            for half in range(2):
                of_sb = pb.tile([128, N], F32, tag=f"of{half}", name="of_sb")
                of.append(of_sb)
            for d in range(HEAD):
                for ci, (c0, c1) in enumerate(CH):
                    cnt = c1 - c0
                    col = (d * 3 + ci) * H
                    pob = ps_tr.tile([H, 128], F32, tag="tr", name="pob")
                    nc.tensor.transpose(pob[:, 0:cnt], oacc[0:cnt, col : col + H],
                                        ident[0:cnt, 0:cnt])
                    nc.scalar.activation(
                        of[d // 2][(d % 2) * H : (d % 2) * H + H, c0:c1],
                        pob[:, 0:cnt], AF.Copy)

            pd = ps_mm.tile([H, N], F32, tag="mm", name="pd")
            nc.tensor.matmul(out=pd[:, :], lhsT=cs_t["d1_wT0"][:, :], rhs=of[0][:, :],
                             start=True, stop=False)
            nc.tensor.matmul(out=pd[:, :], lhsT=cs_t["d1_wT1"][:, :], rhs=of[1][:, :],
                             start=False, stop=True)
            rr_sb = wk.tile([H, N], F32, tag="rr", name="rr_sb")
            nc.scalar.activation(rr_sb[:, :], pd[:, :], AF.Relu,
                                 bias=cs_t["d1_b"][:, 0:1])
            for i in range(P):
                res_block("residual", i, rr_sb[:, :])

            nc.vector.tensor_tensor(rr_sb[:, :], rr_sb[:, :], pout[:, :], op=ALU.add)
            ppr = ps_mm.tile([OUT, N], F32, tag="mm", name="ppr")
            nc.tensor.matmul(out=ppr[:, :], lhsT=cs_t["pred_wT"][:, :],
                             rhs=rr_sb[:, :], start=True, stop=True)
            outp = wk.tile([OUT, N], F32, tag="outp", name="outp")
            nc.scalar.activation(outp[:, :], ppr[:, :], AF.Identity,
                                 bias=cs_t["pred_b"][:, 0:1])
            nc.sync.dma_start(out_d[b], outp[:, :])
    return nc


_CACHED = {}


def _get_nc(reps=1):
    key = ("nc", reps)
    if key not in _CACHED:
        nc = bacc.Bacc("TRN2", target_bir_lowering=False, debug=False,
                       enable_asserts=False, num_devices=NCORES)
        _build_program(nc, reps=reps)
        nc.compile()
        _CACHED[key] = nc
    return _CACHED[key]


def _make_in_maps(x_data, time_id, day_id, params):
    x_data = np.asarray(x_data, np.float32)
    time_id = np.asarray(time_id, np.int32)
    day_id = np.asarray(day_id, np.int32)
    pp = _prep_params(params)
    in_maps = []
    for c in range(NCORES):
        b0, b1 = c * BPC, (c + 1) * BPC
        m = dict(pp)
        m["xb"] = np.ascontiguousarray(x_data[b0:b1])
        m["tdid"] = np.ascontiguousarray(
            np.stack([time_id[b0:b1, -1, :], day_id[b0:b1, -1, :]], 1)
        )
        in_maps.append(m)
    return in_maps


def kernel(x_data, time_id, day_id, params):
    in_maps = _make_in_maps(x_data, time_id, day_id, params)
    nc = _get_nc()
    res = run_bass_kernel_spmd(nc, in_maps, list(range(NCORES)))
    return np.concatenate([res.results[c]["out"] for c in range(NCORES)], 0)
